# revision 1
# baseline (speedup 1.0000x reference)
"""Trainium2 Bass kernel for nn_CrossAttention_46540265619919.

Cross-attention with gene-axis pre-reduction, causal softmax, residual +
LayerNorm.  Full (unsharded) inputs in, full output out; internally sharded
across 8 NeuronCores as (batch b, row-tile pair): core c -> b = c//2, h = c%2.

Causal-skip schedule: the four 128-row L-tiles of a batch are paired
(wide, narrow) so each core's two slots have uniform score widths
(512, 256) while total causal work stays balanced across the pair:
  h=0 -> tiles {3, 0}  (k-extents 512, 128 -> run as 512, 256)
  h=1 -> tiles {2, 1}  (k-extents 384, 256 -> run as 512, 256)
The wide slot streams first so the tail (after the last x_query byte
lands) is only the narrow slot's epilogue.  Fully-masked k-blocks are
never computed; the causal mask is built on-chip from an iota constant
and a tiny per-core row-index threshold vector (no mask DMA).

Self-contained: hardcodes all shapes; no sibling imports.
"""

import os
from contextlib import ExitStack

import numpy as np

import concourse.bass as bass
import concourse.tile as tile
from concourse import bacc, mybir
from concourse.bass_utils import run_bass_kernel_spmd

F32 = mybir.dt.float32
F32R = mybir.dt.float32r
AX = mybir.AxisListType
OP = mybir.AluOpType
AF = mybir.ActivationFunctionType

# Problem shape (fixed).
B, L, K, GT, GC, D = 4, 512, 512, 512, 256, 64
NCORES = 8
LLOC = L // 2          # 256 L-rows per core (two 128-row slots)
KC = K // 128          # 4 k-chunks of 128
SLOTS = ((512, 4), (256, 2))   # (score width, attn k-blocks) per slot
# gene-axis reduction chunks per slot (sum = GT); the last two are small and
# their trees run on different engines (gpsimd then vector) so the stream
# tail stays short.
XQ_CHUNKS = (128, 128, 128, 96, 32)
GC_LOC = GC // 2       # each core of a pair reduces half the key gene axis
MASK_PENALTY = 1.0e9
LN_EPS = 1e-3

LAST_RESULTS = None    # BassKernelResults of the most recent run (for test harness)
_CACHED_NC = None


def _ensure_trace_hook():
    """If NTFF tracing is requested but this image's `antenv` lacks
    `axon_hooks`, synthesize it from trn_boot's ctypes path so
    run_bass_kernel_spmd's trace branch doesn't crash. Best-effort."""
    try:
        import antenv.axon_hooks  # noqa: F401
        return
    except ImportError:
        pass
    try:
        import sys
        import types
        import trn_agent_boot.trn_boot as tb
        import concourse.bass_utils as bu
        hook = tb._ntff_profile_via_ctypes("/opt/axon/libaxon_pjrt.so")
        mod = types.ModuleType("antenv.axon_hooks")
        mod.get_axon_ntff_profile_hook = lambda: hook
        mod.set_axon_ntff_profile_hook = lambda h: None
        sys.modules["antenv.axon_hooks"] = mod
        bu.upload_artifacts = lambda tmpdir: tmpdir  # no fish creds in-container
    except Exception:
        os.environ["BASS_NEVER_TRACE"] = "1"  # fall back: run untraced


def _build_program():
    """Build + compile the per-core SPMD Tile program."""
    nc = bacc.Bacc(
        "TRN2",
        target_bir_lowering=False,
        debug=False,
        num_devices=NCORES,
    )

    xq_d = nc.dram_tensor("xq", [LLOC, GT, D], F32, kind="ExternalInput").ap()
    ck_d = nc.dram_tensor("ck", [K, GC_LOC, D], F32, kind="ExternalInput").ap()
    cv_d = nc.dram_tensor("cv", [K, GT], F32, kind="ExternalInput").ap()
    x_d = nc.dram_tensor("xres", [LLOC, GT], F32, kind="ExternalInput").ap()
    thr_d = nc.dram_tensor("thr", [128, 2], F32, kind="ExternalInput").ap()
    out_d = nc.dram_tensor("out", [LLOC, GT], F32, kind="ExternalOutput").ap()

    with tile.TileContext(nc) as tc, ExitStack() as ctx:
        const = ctx.enter_context(tc.tile_pool(name="const", bufs=1))
        stream = ctx.enter_context(tc.tile_pool(name="stream", bufs=4))
        work = ctx.enter_context(tc.tile_pool(name="work", bufs=2))
        smalls = ctx.enter_context(tc.tile_pool(name="smalls", bufs=2))
        ps_mm = ctx.enter_context(tc.tile_pool(name="ps_mm", bufs=3, space="PSUM"))
        ps_tp = ctx.enter_context(tc.tile_pool(name="ps_tp", bufs=2, space="PSUM"))
        dram = ctx.enter_context(tc.tile_pool(name="dram", bufs=1, space="DRAM"))

        def reduce_gene_axis(eng, t, ng, out_ap):
            """Sum t[128, ng, D] over its gene axis into out_ap[128, D].

            In-place contiguous tensor_tensor halving down to 8 gene rows
            (t[:, 0:n/2] += t[:, n/2:n]), then one short strided reduce.
            `eng` picks the engine; gpsimd lacks free-axis tensor_reduce, so
            there the tree runs down to a single gene row."""
            n = ng
            while n > 8:
                half = n // 2
                eng.tensor_add(t[:, 0:half, :], t[:, 0:half, :], t[:, half:n, :])
                n = half
            eng.tensor_reduce(
                out_ap, t[:, 0:n, :].rearrange("p g d -> p d g"),
                axis=AX.X, op=OP.add,
            )

        # ---- Emission order is engine program order; the sync engine's
        # dma_start issues block head-of-line on their waits, and the DMA
        # queues are FIFO (a transfer lands at ~queue-byte-position / 336GB/s),
        # so every DMA below is placed to keep the queues saturated.
        #
        # The pair k_red exchange (AllReduce, ~3.7 GB/s wire, worse under
        # HBM load) is pulled OFF the critical path via linearity:
        #   scores = q @ k_local^T + q @ (k_total - k_local)^T
        # The local-partial matmuls run per-chunk with no collective
        # dependency; the exchanged total is only needed for one small
        # correction matmul per slot, ~40us later than a direct scheme.

        # Constants first (cheap, off the stream path).
        ones = const.tile([128, 128], F32, tag="ones")
        ident = const.tile([128, 128], F32, tag="ident")
        nc.vector.memset(ones[:], 1.0)
        eps_b = const.tile([128, 1], F32, tag="eps_b")
        nc.vector.memset(eps_b[:], LN_EPS)
        nc.gpsimd.affine_select(
            ident[:], ones[:],
            pattern=[[-1, 128]], base=0, channel_multiplier=1,
            compare_op=OP.is_equal, fill=0.0,
        )
        # k-index iota constant [128, 512] (same in every partition) for the
        # on-chip causal mask, and the per-core row-index thresholds.
        iota_k = const.tile([128, K], F32, tag="iota_k")
        nc.gpsimd.iota(
            iota_k[:], pattern=[[1, K]], base=0, channel_multiplier=0,
            allow_small_or_imprecise_dtypes=True,
        )
        thr_sb = const.tile([128, 2], F32, tag="thr")
        nc.sync.dma_start(thr_sb[:], thr_d[:, :])

        # ---- context_key: reduce the LOCAL gene-half per 128-k chunk.
        kred_in = dram.tile([128, KC, D], F32, tag="kred_in")
        kred_out = dram.tile([128, KC, D], F32, tag="kred_out")
        ck_tiles = [stream.tile([128, 128, D], F32, name=f"ck_t{kc}", tag="stream")
                    for kc in range(KC)]
        for kc in range(KC):
            nc.sync.dma_start(
                ck_tiles[kc][:, 0:GC_LOC, :], ck_d[kc * 128:(kc + 1) * 128, :, :]
            )
        # context_value DMAs ride early in the queue (needed ~halfway in).
        cv_sb = const.tile([128, KC, GT], F32R, tag="cv")
        cv_stages = []
        for kc in range(KC):
            cv_stage = smalls.tile([128, GT], F32, tag="cv_stage", bufs=4)
            nc.sync.dma_start(cv_stage[:], cv_d[kc * 128:(kc + 1) * 128, :])
            cv_stages.append(cv_stage)
        kreds = []
        for kc in range(KC):
            k_red = smalls.tile([128, D], F32, tag="k_red", bufs=4)
            reduce_gene_axis(nc.vector, ck_tiles[kc], GC_LOC, k_red[:])
            kreds.append(k_red)
        for kc in range(KC):
            nc.scalar.copy(cv_sb[:, kc, :], cv_stages[kc][:])
        # kred partial writes to DRAM (on the gpsimd DMA queue, bypassing
        # the stream FIFO) + the pair AllReduce.
        for kc in range(KC):
            nc.gpsimd.dma_start(kred_in[:, kc, :], kreds[kc][:])
        nc.gpsimd.collective_compute(
            "AllReduce", OP.add,
            replica_groups=[[2 * b, 2 * b + 1] for b in range(B)],
            ins=[kred_in.opt()], outs=[kred_out.opt()],
        )
        # causal mask bits on gpsimd AFTER the collective trigger (slow
        # there, but the engine is idle and they are needed late).
        bits_sb = []
        for s, (W, _) in enumerate(SLOTS):
            bt = const.tile([128, W], F32, tag=f"bits{s}")
            nc.gpsimd.tensor_scalar(
                bt[:], iota_k[:, 0:W], thr_sb[:, s:s + 1], -MASK_PENALTY,
                op0=OP.is_gt, op1=OP.mult,
            )
            bits_sb.append(bt)

        def emit_chunk(s, gc, g0, ng, ps_s, W):
            """One x_query gene chunk: DMA, tree, transpose, and a scores
            matmul against k_totT accumulating in PSUM.  The matmuls simply
            queue on the PE until the collective result lands (~134us); the
            scores are not consumed until much later, and no vector op may
            depend on the collective (the scheduler would hoist it into the
            tree stream and stall the DMA pipeline)."""
            lsl = slice(s * 128, (s + 1) * 128)
            t = stream.tile([128, 128, D], F32, tag="stream")
            nc.sync.dma_start(t[:, 0:ng, :], xq_d[lsl, g0:g0 + ng, :])
            qp = smalls.tile([128, D], F32, tag="qp", bufs=8)
            reduce_gene_axis(nc.vector, t[:, 0:ng, :], ng, qp[:])
            tq = ps_tp.tile([D, 128], F32, tag="tpose_q", bufs=3)
            nc.tensor.transpose(tq[:], qp[:], ident[:])
            qT = smalls.tile([D, 128], F32, tag="qT", bufs=10)
            nc.scalar.copy(qT[:], tq[:])
            nc.tensor.matmul(
                ps_s[:, 0:W], qT[:], k_totT[:, 0:W],
                start=(gc == 0), stop=(gc == len(XQ_CHUNKS) - 1),
            )

        NCH = len(XQ_CHUNKS)
        G0 = [sum(XQ_CHUNKS[:i]) for i in range(NCH)]
        W0, NKC0 = SLOTS[0]
        W1, NKC1 = SLOTS[1]

        # AllReduce result readback rides the gpsimd DMA queue, bypassing
        # the stream FIFO; it lands right when the collective completes.
        kred_sb = smalls.tile([128, KC, D], F32, tag="kred_sb")
        nc.gpsimd.dma_start(kred_sb[:], kred_out[:])
        k_totT = const.tile([64, K], F32, tag="k_totT")
        for kc in range(KC):
            tp = ps_tp.tile([D, 128], F32, tag="tpose")
            nc.tensor.transpose(tp[:], kred_sb[:, kc, :], ident[:])
            nc.scalar.copy(k_totT[:, kc * 128:(kc + 1) * 128], tp[:])

        # ---- slot 0 stream.
        x_t0 = smalls.tile([128, GT], F32, tag="x_t")
        nc.sync.dma_start(x_t0[:], x_d[0:128, :])
        ps_s0 = ps_mm.tile([128, K], F32, tag="mm")
        for gc in range(NCH):
            emit_chunk(0, gc, G0[gc], XQ_CHUNKS[gc], ps_s0, W0)

        # ---- slot 1 stream.
        ps_s1 = ps_mm.tile([128, K], F32, tag="mm")
        for gc in range(2):
            emit_chunk(1, gc, G0[gc], XQ_CHUNKS[gc], ps_s1, W1)

        def emit_epilogue(s, W, nkc, ps_s, x_t, bits):
            lsl = slice(s * 128, (s + 1) * 128)
            # masked scores in SBUF: s = scores + bits  (bits is 0 / -1e9)
            s_sb = work.tile([128, K], F32, tag="s_sb")
            nc.vector.scalar_tensor_tensor(
                s_sb[:, 0:W], bits[:], 1.0, ps_s[:, 0:W],
                op0=OP.mult, op1=OP.add,
            )
            negmax = smalls.tile([128, 1], F32, tag="negmax")
            nc.vector.tensor_reduce(
                negmax[:], s_sb[:, 0:W], axis=AX.X, op=OP.max, negate=True
            )
            w = work.tile([128, K], F32, tag="w")
            denom = smalls.tile([128, 1], F32, tag="denom")
            nc.scalar.activation(
                w[:, 0:W], s_sb[:, 0:W], AF.Exp, bias=negmax[:], scale=1.0,
                accum_out=denom[:],
            )
            recip = smalls.tile([128, 1], F32, tag="recip")
            nc.vector.reciprocal(recip[:], denom[:])
            # w^T via TensorE transpose; only causally-live k-blocks exist.
            wT = work.tile([128, KC, 128], F32R, tag="wT")
            for kc in range(nkc):
                tw = ps_tp.tile([128, 128], F32, tag="tpose")
                nc.tensor.transpose(tw[:], w[:, kc * 128:(kc + 1) * 128], ident[:])
                nc.scalar.copy(wT[:, kc, :], tw[:])
            ps_a = ps_mm.tile([128, GT], F32, tag="mm")
            for kc in range(nkc):
                nc.tensor.matmul(
                    ps_a[:], wT[:, kc, :], cv_sb[:, kc, :],
                    start=(kc == 0), stop=(kc == nkc - 1),
                )
            # y = attn * recip + x
            y = work.tile([128, GT], F32, tag="y")
            nc.vector.scalar_tensor_tensor(
                y[:], ps_a[:], recip[:], x_t[:], op0=OP.mult, op1=OP.add
            )
            stats = smalls.tile([128, 6], F32, tag="stats")
            nc.vector.bn_stats(stats[:], y[:])
            mv = smalls.tile([128, 2], F32, tag="mv")
            nc.vector.bn_aggr(mv[:], stats[:])
            std = smalls.tile([128, 1], F32, tag="std")
            nc.scalar.activation(std[:], mv[:, 1:2], AF.Sqrt, bias=eps_b[:], scale=1.0)
            rstd = smalls.tile([128, 1], F32, tag="rstd")
            nc.vector.reciprocal(rstd[:], std[:])
            o_t = work.tile([128, GT], F32, tag="o_t")
            nc.vector.tensor_scalar(
                o_t[:], y[:], mv[:, 0:1], rstd[:], op0=OP.subtract, op1=OP.mult
            )
            return o_t

        # slot 1's middle chunks; slot 0's epilogue is emitted between the
        # fourth and fifth so its first vector op reaches the in-order vector
        # engine only after the collective (~135us) is surely done — placing
        # it earlier blocks the vector stream and starves the DMA queues.
        for gc in range(2, 4):
            emit_chunk(1, gc, G0[gc], XQ_CHUNKS[gc], ps_s1, W1)
        o_t0 = emit_epilogue(0, W0, NKC0, ps_s0, x_t0, bits_sb[0])
        for gc in range(4, NCH):
            emit_chunk(1, gc, G0[gc], XQ_CHUNKS[gc], ps_s1, W1)
        # slot-1 residual rows land after the last xq chunk (needed latest).
        x_t1 = smalls.tile([128, GT], F32, tag="x_t")
        nc.sync.dma_start(x_t1[:], x_d[128:256, :])
        nc.sync.dma_start(out_d[0:128, :], o_t0[:])
        o_t1 = emit_epilogue(1, W1, NKC1, ps_s1, x_t1, bits_sb[1])
        nc.sync.dma_start(out_d[128:256, :], o_t1[:])

    nc.compile()
    return nc


def _get_nc():
    global _CACHED_NC
    if _CACHED_NC is None:
        _CACHED_NC = _build_program()
    return _CACHED_NC


def _tiles_for(h: int) -> tuple[int, int]:
    """Row-tile indices (slot0, slot1) handled by pair-half h."""
    return 3 - h, h


def kernel(x, x_query, context_key, context_value, gamma, beta):
    global LAST_RESULTS
    x = np.asarray(x, np.float32)
    x_query = np.asarray(x_query, np.float32)
    context_key = np.asarray(context_key, np.float32)
    context_value = np.asarray(context_value, np.float32)
    gamma = np.asarray(gamma, np.float32)
    beta = np.asarray(beta, np.float32)

    nc = _get_nc()
    p_idx = np.arange(128, dtype=np.float32)
    in_maps = []
    for c in range(NCORES):
        b, h = c // 2, c % 2
        t0, t1 = _tiles_for(h)
        r0 = slice(t0 * 128, (t0 + 1) * 128)
        r1 = slice(t1 * 128, (t1 + 1) * 128)
        thr = np.empty((128, 2), np.float32)
        thr[:, 0] = t0 * 128 + p_idx
        thr[:, 1] = t1 * 128 + p_idx
        in_maps.append({
            "xq": np.concatenate([x_query[b, r0], x_query[b, r1]]),
            "ck": np.ascontiguousarray(context_key[b, :, h * GC_LOC:(h + 1) * GC_LOC]),
            "cv": np.ascontiguousarray(context_value[b]),
            "xres": np.concatenate([x[b, r0], x[b, r1]]),
            "thr": thr,
        })

    if os.environ.get("KERNEL_TRACE") or os.environ.get("BASS_TRACE"):
        _ensure_trace_hook()
    res = run_bass_kernel_spmd(
        nc,
        in_maps,
        core_ids=list(range(NCORES)),
        trace=bool(os.environ.get("KERNEL_TRACE")),
    )
    LAST_RESULTS = res

    out = np.empty((B, L, GT), np.float32)
    for c, r in enumerate(res.results):
        b, h = c // 2, c % 2
        t0, t1 = _tiles_for(h)
        out[b, t0 * 128:(t0 + 1) * 128] = r["out"][0:128]
        out[b, t1 * 128:(t1 + 1) * 128] = r["out"][128:256]
    # LN affine (gamma/beta broadcast over the last axis) applied on host.
    out = out * gamma + beta
    return out.astype(np.float32)

